# revision 1
# baseline (speedup 1.0000x reference)
"""Trainium2 Bass kernel for nn_MiNer2 (gnn_message_passing), 8-core SPMD.

Strategy (tuned to this HW's indirect-DMA primitive: one row per partition
per instruction, i.e. 128 gathered rows / ~1.2us Pool-engine instruction):
  - dst-node data-parallel over 8 cores (2500 block0/block1 dst rows per core)
  - src pools built once on device ([P,1] row gathers from entity), sharded
    across cores and AllGathered
  - block1 gathers fused: SA[i] = [S1[i] | agg[i]] (20000 x 400 f32) so one
    instruction fetches both mailbox halves for an edge
  - block0 relation sum is linear -> computed as a PE matmul against a
    host-prepared per-dst relation-count matrix (C @ rel2)
  - fc via PE (transpose -> matmul), relu fused into ACT PSUM evacuation
  - parameter-free MHA pooling on DVE in bf16 with (h, dh, l) packed layout
"""
import sys
import os

sys.path.insert(0, "/opt/trn_rl_repo")

import numpy as np

import concourse.bass as bass
import concourse.bacc as bacc
import concourse.tile as tile
import concourse.mybir as mybir
from concourse.bass import IndirectOffsetOnAxis
from concourse.bass_utils import run_bass_kernel_spmd
from concourse.masks import make_identity

F32 = mybir.dt.float32
BF16 = mybir.dt.bfloat16
I32 = mybir.dt.int32


class Cfg:
    def __init__(self, n_ent=100000, n_rels=500, n_types=50, n_src0=50000,
                 n1=20000, n2=20000, d=20, hidden=200, heads=5, beta=0.3,
                 n_cores=8):
        self.n_ent = n_ent
        self.n_rels = n_rels
        self.n_types = n_types
        self.n_src0 = n_src0
        self.n1 = n1
        self.n2 = n2
        self.d = d
        self.h = hidden
        self.heads = heads
        self.dh = 10
        self.beta = beta
        self.n_cores = n_cores
        self.nt = n_types
        self.l = d + 1
        self.nr2 = 2 * n_rels              # signed relation table rows
        assert n1 % n_cores == 0 and n2 % n_cores == 0
        self.sh1 = n1 // n_cores
        self.sh2 = n2 // n_cores
        self.t0 = -(-self.sh1 // 128)
        self.t1 = -(-self.sh2 // 128)
        assert n_src0 % n_cores == 0
        self.bsh0 = n_src0 // n_cores      # S0 rows built per core
        self.bt0 = -(-self.bsh0 // 128)
        self.bsh1 = n1 // n_cores
        self.bt1 = -(-self.bsh1 // 128)
        self.rk = -(-self.nr2 // 8)        # rel K-chunk (125 for 1000)
        self.fsplit = min(128, hidden)


CFG = Cfg()


def prep_core_inputs(cfg, entity, relation, fc_w, fc_b, ids0, edge_src0,
                     etype0, ids1, edge_src1, etype1, core):
    """Host-side input prep for one core: sharding + index relayout +
    per-dst relation-count histogram (index statistics only)."""
    c = core
    d = cfg.d

    def shard_edges(arr, n_dst, sh, ntile):
        a = np.asarray(arr, np.int32).reshape(n_dst, d)[c * sh:(c + 1) * sh]
        pad = ntile * 128 - sh
        if pad:
            a = np.concatenate([a, np.zeros((pad, d), a.dtype)], 0)
        return a.reshape(ntile, 128, d)

    e0 = shard_edges(edge_src0, cfg.n1, cfg.sh1, cfg.t0)
    t0 = shard_edges(etype0, cfg.n1, cfg.sh1, cfg.t0)
    e1 = shard_edges(edge_src1, cfg.n2, cfg.sh2, cfg.t1)
    t1 = shard_edges(etype1, cfg.n2, cfg.sh2, cfg.t1)

    # per-dst signed-relation count matrix for block0, transposed to
    # [T0, nr2, 128] (K-major for the PE), bf16 (counts <= d, exact)
    counts = np.zeros((cfg.t0, 128, cfg.nr2), np.float32)
    tiles_idx = np.arange(cfg.t0)[:, None, None]
    rows_idx = np.arange(128)[None, :, None]
    np.add.at(counts, (tiles_idx, rows_idx, t0), 1.0)
    c0rT = np.ascontiguousarray(counts.transpose(0, 2, 1), np.float32)

    def build_ids(idvec, nrows, ntiles):
        padded = np.zeros(ntiles * 128, np.int32)
        padded[:nrows] = np.asarray(idvec, np.int32)
        return padded.reshape(ntiles, 128, 1)

    ids0 = np.asarray(ids0)
    ids1 = np.asarray(ids1)
    ids0_sh = ids0[c * cfg.bsh0:(c + 1) * cfg.bsh0]
    ids1_sh = ids1[c * cfg.bsh1:(c + 1) * cfg.bsh1]

    return dict(
        entity=np.ascontiguousarray(entity, np.float32),
        relation=np.ascontiguousarray(relation, np.float32),
        fc_w=np.ascontiguousarray(fc_w, np.float32),
        fc_b=np.ascontiguousarray(fc_b, np.float32).reshape(cfg.nt, 1),
        i_e0=e0, i_t0=t0, i_e1=e1, i_t1=t1,
        c0rT=c0rT,
        b_ids0=build_ids(ids0_sh, cfg.bsh0, cfg.bt0),
        b_ids1=build_ids(ids1_sh, cfg.bsh1, cfg.bt1),
    )


def _bc(ap_obj, dims):
    """Manual AP with given free dims (for step-0 broadcasts)."""
    return bass.AP(ap_obj.tensor, ap_obj.offset, [ap_obj.ap[0]] + dims)


def build_program(cfg):
    nc = bacc.Bacc("TRN2", target_bir_lowering=False, debug=False,
                   num_devices=cfg.n_cores)
    d, h, nt, L = cfg.d, cfg.h, cfg.nt, cfg.l
    H5, DH = cfg.heads, cfg.dh
    fA = cfg.fsplit
    fB = h - fA
    groups = [list(range(g, min(g + 4, L))) for g in range(0, L, 4)]
    ncols = L * 128
    nch = [(j, min(512, ncols - j)) for j in range(0, ncols, 512)]
    pos = [(i, min(125, cfg.n_rels - i)) for i in range(0, cfg.n_rels, 125)]
    rchunks = pos + [(cfg.n_rels + i, r) for i, r in pos]

    ein = lambda name, shape, dt: nc.dram_tensor(name, shape, dt,
                                                 kind="ExternalInput").ap()
    entity = ein("entity", [cfg.n_ent, h], F32)
    relation = ein("relation", [cfg.n_rels, h], F32)
    fc_w = ein("fc_w", [nt, h], F32)
    fc_b = ein("fc_b", [nt, 1], F32)
    i_e0 = ein("i_e0", [cfg.t0, 128, d], I32)
    i_t0 = ein("i_t0", [cfg.t0, 128, d], I32)
    i_e1 = ein("i_e1", [cfg.t1, 128, d], I32)
    i_t1 = ein("i_t1", [cfg.t1, 128, d], I32)
    c0rT = ein("c0rT", [cfg.t0, cfg.nr2, 128], F32)
    b_ids0 = ein("b_ids0", [cfg.bt0, 128, 1], I32)
    b_ids1 = ein("b_ids1", [cfg.bt1, 128, 1], I32)

    out = nc.dram_tensor("out", [cfg.sh2, nt], F32, kind="ExternalOutput").ap()

    with tile.TileContext(nc) as tc:
        with tc.tile_pool(name="dram", bufs=1, space="DRAM") as dram, \
             tc.tile_pool(name="const", bufs=1) as cst, \
             tc.tile_pool(name="mail", bufs=2) as mailp, \
             tc.tile_pool(name="xt", bufs=2) as xtp, \
             tc.tile_pool(name="small", bufs=2) as smp, \
             tc.tile_pool(name="psT", bufs=2, space="PSUM") as psTp, \
             tc.tile_pool(name="psF", bufs=2, space="PSUM") as psFp, \
             tc.tile_pool(name="psP", bufs=2, space="PSUM") as psPp:

            # ---------------- DRAM internals ----------------
            S0 = dram.tile([cfg.n_src0, h], BF16)         # entity[ids0]
            SA = dram.tile([cfg.n1, 2 * h], BF16)         # [S1 | agg]
            rel2 = dram.tile([cfg.nr2, h], BF16)          # [relation; -relation]
            aggp = dram.tile([cfg.n2, h], BF16)
            shS0 = dram.tile([cfg.bsh0, h], BF16)
            shS1 = dram.tile([cfg.bsh1, h], BF16)
            shAgg = dram.tile([cfg.sh1, h], BF16)
            S1g = dram.tile([cfg.n1, h], BF16)

            # ---------------- constants ----------------
            id_f32 = cst.tile([128, 128], F32)
            make_identity(nc, id_f32[:])
            id_bf = cst.tile([128, 128], BF16)
            nc.vector.tensor_copy(id_bf[:], id_f32[:])

            fcw_sb = cst.tile([nt, h], F32)
            nc.sync.dma_start(fcw_sb[:], fc_w[:, :])
            fcb_sb = cst.tile([nt, 1], F32)
            nc.sync.dma_start(fcb_sb[:], fc_b[:, :])
            fcw_a = cst.tile([fA, nt], BF16)
            fcw_b = cst.tile([fB, nt], BF16)
            pw = psFp.tile([128, 512], F32, tag="psF")
            nc.tensor.transpose(pw[:fA, :nt], fcw_sb[:, 0:fA], id_f32[:nt, :nt])
            nc.vector.tensor_copy(fcw_a[:], pw[:fA, :nt])
            pw2 = psFp.tile([128, 512], F32, tag="psF")
            nc.tensor.transpose(pw2[:fB, :nt], fcw_sb[:, fA:h], id_f32[:nt, :nt])
            nc.vector.tensor_copy(fcw_b[:], pw2[:fB, :nt])

            # rel2 build: [relation; -relation], plus SBUF chunks (K-major,
            # 125 rows each, positives then negatives) for the block0 matmul
            pos_chunks = []
            neg_chunks = []
            for i in range(0, cfg.n_rels, 125):
                rows = min(125, cfg.n_rels - i)
                rl = cst.tile([128, h], F32, name=f"relsb{i}")
                nc.sync.dma_start(rl[:rows, :], relation[i:i + rows, :])
                rlb = smp.tile([128, h], BF16, tag="relbf")
                nc.vector.tensor_copy(rlb[:rows, :], rl[:rows, :])
                nc.sync.dma_start(rel2[i:i + rows, :], rlb[:rows, :])
                rln = cst.tile([128, h], F32, name=f"relsbn{i}")
                nc.vector.tensor_scalar_mul(rln[:rows, :], rl[:rows, :], -1.0)
                rlnb = smp.tile([128, h], BF16, tag="relbf")
                nc.vector.tensor_copy(rlnb[:rows, :], rln[:rows, :])
                nc.sync.dma_start(rel2[cfg.n_rels + i:cfg.n_rels + i + rows, :],
                                  rlnb[:rows, :])
                pos_chunks.append((rl, rows))
                neg_chunks.append((rln, rows))
            rel_chunks = pos_chunks + neg_chunks

            # ---------------- src pool builds (sharded) ----------------
            def build_pool(ids_arr, ntiles, nrows, shard):
                for t in range(ntiles):
                    rows = min(128, nrows - t * 128)
                    it = smp.tile([128, 1], I32, tag="bidx")
                    nc.sync.dma_start(it[:], ids_arr[t, :, :])
                    g = smp.tile([128, h], F32, tag="brow")
                    nc.gpsimd.indirect_dma_start(
                        out=g[:], out_offset=None, in_=entity[:, :],
                        in_offset=IndirectOffsetOnAxis(ap=it[:], axis=0))
                    gb = smp.tile([128, h], BF16, tag="browb")
                    nc.vector.tensor_copy(gb[:], g[:])
                    nc.sync.dma_start(shard[t * 128:t * 128 + rows, :],
                                      gb[:rows, :])

            build_pool(b_ids0, cfg.bt0, cfg.bsh0, shS0)
            build_pool(b_ids1, cfg.bt1, cfg.bsh1, shS1)

            grp = [list(range(cfg.n_cores))]
            nc.gpsimd.collective_compute("AllGather", mybir.AluOpType.bypass,
                                         replica_groups=grp, ins=[shS0.opt()],
                                         outs=[S0.opt()])
            nc.gpsimd.collective_compute("AllGather", mybir.AluOpType.bypass,
                                         replica_groups=grp, ins=[shS1.opt()],
                                         outs=[S1g.opt()])
            # copy S1g into SA[:, 0:h] (strided DRAM->DRAM, <16k descs each)
            hw_ = cfg.n1 // 2
            nc.sync.dma_start(SA[0:hw_, 0:h], S1g[0:hw_, :])
            nc.sync.dma_start(SA[hw_:cfg.n1, 0:h], S1g[hw_:cfg.n1, :])

            # ---------------- block 0 ----------------
            for t in range(cfg.t0):
                rows = min(128, cfg.sh1 - t * 128)
                ie = smp.tile([128, d], I32, tag="ie0")
                nc.sync.dma_start(ie[:], i_e0[t, :, :])
                mb = mailp.tile([128, d * h], BF16, tag="mailSA", bufs=3)
                for k in range(d):
                    nc.gpsimd.indirect_dma_start(
                        out=mb[:, k * h:(k + 1) * h], out_offset=None,
                        in_=S0[:, :],
                        in_offset=IndirectOffsetOnAxis(ap=ie[:, k:k + 1], axis=0))
                # relation sum via count matmul: psr = C^T.T @ rel2_sb
                psr = psPp.tile([128, 200], F32, tag="psP")
                for ci, (r0, rn_) in enumerate(rchunks):
                    ct = smp.tile([128, 128], F32, tag="c0chunk")
                    nc.sync.dma_start(ct[:rn_, :], c0rT[t, r0:r0 + rn_, :])
                    rtile, rrows = rel_chunks[ci]
                    assert rrows == rn_
                    nc.tensor.matmul(psr[:, :h], lhsT=ct[:rn_, :],
                                     rhs=rtile[:rn_, :],
                                     start=(ci == 0), stop=(ci == len(rchunks) - 1))
                su = smp.tile([128, h], F32, tag="msum")
                mrv = mb[:].rearrange("p (k e) -> p e k", k=d, e=h)
                nc.vector.tensor_reduce(su[:], mrv, axis=mybir.AxisListType.X,
                                        op=mybir.AluOpType.add)
                abf = smp.tile([128, h], F32, tag="aggf")
                nc.vector.tensor_add(abf[:], su[:], psr[:, :h])
                ab = smp.tile([128, h], BF16, tag="aggbf")
                nc.vector.tensor_scalar_mul(ab[:], abf[:], 1.0 / d)
                nc.sync.dma_start(shAgg[t * 128:t * 128 + rows, :], ab[:rows, :])

            nc.gpsimd.collective_compute("AllGather", mybir.AluOpType.bypass,
                                         replica_groups=grp, ins=[shAgg.opt()],
                                         outs=[aggp.opt()])
            nc.sync.dma_start(SA[0:hw_, h:2 * h], aggp[0:hw_, :])
            nc.sync.dma_start(SA[hw_:cfg.n1, h:2 * h], aggp[hw_:cfg.n1, :])

            # ---------------- block 1 ----------------
            scale_q = (1.0 / L) * (DH ** -0.5)
            for t in range(cfg.t1):
                rows = min(128, cfg.sh2 - t * 128)
                ie = smp.tile([128, d], I32, tag="ie1")
                it1 = smp.tile([128, d], I32, tag="it1")
                nc.sync.dma_start(ie[:], i_e1[t, :, :])
                nc.sync.dma_start(it1[:], i_t1[t, :, :])

                sa = mailp.tile([128, d * 2 * h], BF16, tag="mailSA", bufs=3)
                rl = mailp.tile([128, d * h], BF16, tag="mailR", bufs=5)
                for k in range(d):
                    nc.gpsimd.indirect_dma_start(
                        out=sa[:, k * 2 * h:(k + 1) * 2 * h], out_offset=None,
                        in_=SA[:, :],
                        in_offset=IndirectOffsetOnAxis(ap=ie[:, k:k + 1], axis=0))
                    nc.gpsimd.indirect_dma_start(
                        out=rl[:, k * h:(k + 1) * h], out_offset=None,
                        in_=rel2[:, :],
                        in_offset=IndirectOffsetOnAxis(ap=it1[:, k:k + 1], axis=0))

                # msg/aggm halves += rel
                rv = rl[:].rearrange("p (k e) -> p k e", k=d, e=h)
                sav = sa[:].rearrange("p (k s e) -> p k s e", k=d, s=2, e=h)
                nc.vector.tensor_add(sav[:, :, 0, :], sav[:, :, 0, :], rv)
                nc.vector.tensor_add(sav[:, :, 1, :], sav[:, :, 1, :], rv)

                p_tiles = []
                for s in range(2):          # 0 = msg, 1 = aggm
                    # mean slot
                    su32 = smp.tile([128, h], F32, tag="msum")
                    mrv = sa[:].rearrange("p (k s e) -> p s e k", k=d, s=2, e=h)[:, s, :, :]
                    nc.vector.tensor_reduce(su32[:], mrv, axis=mybir.AxisListType.X,
                                            op=mybir.AluOpType.add)
                    su = smp.tile([128, h], BF16, tag="msumb")
                    nc.vector.tensor_copy(su[:], su32[:])

                    xA = xtp.tile([fA, ncols], BF16, tag="xA")
                    xB = xtp.tile([fB, ncols], BF16, tag="xB")
                    for g in groups:
                        pst = psTp.tile([128, 1024], BF16, tag="psT")
                        for j, l in enumerate(g):
                            if l < d:
                                c0 = l * 2 * h + s * h
                                srcA = sa[:, c0:c0 + fA]
                                srcB = sa[:, c0 + fA:c0 + h]
                            else:
                                srcA = su[:, 0:fA]
                                srcB = su[:, fA:h]
                            nc.tensor.transpose(pst[:, 256 * j:256 * j + 128],
                                                srcA, id_bf[:128, :128])
                            nc.tensor.transpose(
                                pst[:fB, 256 * j + 128:256 * j + 256],
                                srcB, id_bf[:128, :128])
                        n_g = len(g)
                        l0 = g[0]
                        sc = (1.0 / d) if l0 == d else 1.0
                        inA = pst[:].rearrange("p (b c) -> p b c", b=4, c=256)[:, 0:n_g, 0:128]
                        outA = xA[:].rearrange("f (l n) -> f l n", l=L, n=128)[:, l0:l0 + n_g, :]
                        nc.scalar.activation(outA, inA,
                                             mybir.ActivationFunctionType.Relu,
                                             scale=sc)
                        inB = pst[:fB].rearrange("p (b c) -> p b c", b=4, c=256)[:, 0:n_g, 128:256]
                        outB = xB[:].rearrange("f (l n) -> f l n", l=L, n=128)[:, l0:l0 + n_g, :]
                        nc.scalar.activation(outB, inB,
                                             mybir.ActivationFunctionType.Relu,
                                             scale=sc)

                    pf = smp.tile([nt, ncols], BF16, tag="pf")
                    for j0, w in nch:
                        psf = psFp.tile([128, 512], F32, tag="psF")
                        nc.tensor.matmul(psf[:nt, :w], lhsT=fcw_a[:],
                                         rhs=xA[:, j0:j0 + w], start=True,
                                         stop=False)
                        nc.tensor.matmul(psf[:nt, :w], lhsT=fcw_b[:],
                                         rhs=xB[:, j0:j0 + w], start=False,
                                         stop=True)
                        nc.scalar.activation(pf[:, j0:j0 + w], psf[:nt, :w],
                                             mybir.ActivationFunctionType.Identity,
                                             bias=fcb_sb[:])

                    p_s = smp.tile([128, L * nt], BF16, tag=f"p{s}")
                    for g in groups:
                        psp = psPp.tile([128, 200], BF16, tag="psP")
                        for j, l in enumerate(g):
                            nc.tensor.transpose(psp[:, nt * j:nt * j + nt],
                                                pf[:, 128 * l:128 * l + 128],
                                                id_bf[:nt, :nt])
                        n_g = len(g)
                        l0 = g[0]
                        inP = psp[:].rearrange("p (b c) -> p b c", b=4, c=nt)[:, 0:n_g, :]
                        outP = p_s[:].rearrange("p (c l) -> p l c", c=nt, l=L)[:, l0:l0 + n_g, :]
                        nc.vector.tensor_copy(outP, inP)
                    p_tiles.append(p_s)

                # ---- MHA pooling + combine ----
                outs_f = []
                for p_s in p_tiles:
                    pap = p_s[:]
                    pv = pap.rearrange("p (h dd l) -> p h dd l", h=H5, dd=DH, l=L)
                    q32 = smp.tile([128, nt], F32, tag="q32")
                    qv = pap.rearrange("p (c l) -> p c l", c=nt, l=L)
                    nc.vector.tensor_reduce(q32[:], qv, axis=mybir.AxisListType.X,
                                            op=mybir.AluOpType.add)
                    qs = smp.tile([128, nt], BF16, tag="qs")
                    nc.vector.tensor_scalar_mul(qs[:], q32[:], scale_q)
                    sc1 = smp.tile([128, L * nt], BF16, tag="sc1")
                    sc1v = sc1[:].rearrange("p (h dd l) -> p h dd l", h=H5, dd=DH, l=L)
                    qsv = _bc(qs[:], [[DH, H5], [1, DH], [0, L]])
                    nc.vector.tensor_mul(sc1v, pv, qsv)
                    scor = smp.tile([128, H5 * L], F32, tag="scor")
                    sc1r = sc1[:].rearrange("p (h dd l) -> p h l dd", h=H5, dd=DH, l=L)
                    scv = scor[:].rearrange("p (h l) -> p h l", h=H5, l=L)
                    nc.vector.tensor_reduce(scv, sc1r, axis=mybir.AxisListType.X,
                                            op=mybir.AluOpType.add)
                    smax = smp.tile([128, H5], F32, tag="smax")
                    nc.vector.tensor_reduce(smax[:], scv, axis=mybir.AxisListType.X,
                                            op=mybir.AluOpType.max)
                    smaxb = _bc(smax[:], [[1, H5], [0, L]])
                    nc.vector.tensor_sub(scv, scv, smaxb)
                    nc.scalar.activation(scor[:], scor[:],
                                         mybir.ActivationFunctionType.Exp)
                    ssum = smp.tile([128, H5], F32, tag="ssum")
                    nc.vector.tensor_reduce(ssum[:], scv, axis=mybir.AxisListType.X,
                                            op=mybir.AluOpType.add)
                    rinv = smp.tile([128, H5], F32, tag="rinv")
                    nc.vector.reciprocal(rinv[:], ssum[:])
                    sc2 = smp.tile([128, L * nt], BF16, tag="sc2")
                    sc2v = sc2[:].rearrange("p (h dd l) -> p h dd l", h=H5, dd=DH, l=L)
                    scb = _bc(scor[:], [[L, H5], [0, DH], [1, L]])
                    nc.vector.tensor_mul(sc2v, pv, scb)
                    asum = smp.tile([128, nt], F32, tag="asum")
                    sc2r = sc2[:].rearrange("p (c l) -> p c l", c=nt, l=L)
                    nc.vector.tensor_reduce(asum[:], sc2r, axis=mybir.AxisListType.X,
                                            op=mybir.AluOpType.add)
                    of = smp.tile([128, nt], F32, tag="of")
                    rinvb = _bc(rinv[:], [[1, H5], [0, DH]])
                    nc.vector.tensor_mul(of[:], asum[:], rinvb)
                    outs_f.append(of)

                pred = smp.tile([128, nt], F32, tag="pred")
                nc.vector.tensor_scalar_mul(pred[:], outs_f[1][:], 1.0 - cfg.beta)
                nc.vector.scalar_tensor_tensor(
                    pred[:], outs_f[0][:], cfg.beta, pred[:],
                    op0=mybir.AluOpType.mult, op1=mybir.AluOpType.add)
                osb = smp.tile([128, nt], F32, tag="osb")
                nc.scalar.activation(osb[:], pred[:],
                                     mybir.ActivationFunctionType.Sigmoid)
                nc.sync.dma_start(out[t * 128:t * 128 + rows, :], osb[:rows, :])

    nc.compile()
    return nc


_CACHE = {}


def _get_program():
    if "nc" not in _CACHE:
        _CACHE["nc"] = build_program(CFG)
    return _CACHE["nc"]


def kernel(entity, relation, fc_w, fc_b, ids0, edge_src0, etype0, ids1,
           edge_src1, etype1):
    cfg = CFG
    nc = _get_program()
    in_maps = [
        prep_core_inputs(cfg, entity, relation, fc_w, fc_b, ids0, edge_src0,
                         etype0, ids1, edge_src1, etype1, c)
        for c in range(cfg.n_cores)
    ]
    res = run_bass_kernel_spmd(nc, in_maps, core_ids=list(range(cfg.n_cores)),
                               trace=False)
    out = np.concatenate([res.results[c]["out"] for c in range(cfg.n_cores)],
                         axis=0)
    return out.astype(np.float32)



# revision 19
# speedup vs baseline: 1.1469x; 1.1469x over previous
"""Trainium2 Bass kernel for nn_MiNer2 (gnn_message_passing), 8-core SPMD.

Strategy (tuned to this HW's indirect-DMA primitive: one row per partition
per instruction, i.e. 128 gathered rows / ~1.2us Pool-engine instruction):
  - dst-node data-parallel over 8 cores (2500 block0/block1 dst rows per core)
  - src pools built once on device ([P,1] row gathers from entity), sharded
    across cores and AllGathered
  - block1 gathers fused: SA[i] = [S1[i] | agg[i]] (20000 x 400 f32) so one
    instruction fetches both mailbox halves for an edge
  - block0 relation sum is linear -> computed as a PE matmul against a
    host-prepared per-dst relation-count matrix (C @ rel2)
  - fc via PE (transpose -> matmul), relu fused into ACT PSUM evacuation
  - parameter-free MHA pooling on DVE in bf16 with (h, dh, l) packed layout
"""
import sys
import os

sys.path.insert(0, "/opt/trn_rl_repo")

import numpy as np

import concourse.bass as bass
import concourse.bacc as bacc
import concourse.tile as tile
import concourse.mybir as mybir
from concourse.bass import IndirectOffsetOnAxis
from concourse.bass_utils import run_bass_kernel_spmd
from concourse.masks import make_identity

F32 = mybir.dt.float32
BF16 = mybir.dt.bfloat16
I32 = mybir.dt.int32


class Cfg:
    def __init__(self, n_ent=100000, n_rels=500, n_types=50, n_src0=50000,
                 n1=20000, n2=20000, d=20, hidden=200, heads=5, beta=0.3,
                 n_cores=8):
        self.n_ent = n_ent
        self.n_rels = n_rels
        self.n_types = n_types
        self.n_src0 = n_src0
        self.n1 = n1
        self.n2 = n2
        self.d = d
        self.h = hidden
        self.heads = heads
        self.dh = 10
        self.beta = beta
        self.n_cores = n_cores
        self.nt = n_types
        self.l = d + 1
        self.nr2 = 2 * n_rels              # signed relation table rows
        assert n1 % n_cores == 0 and n2 % n_cores == 0
        self.sh1 = n1 // n_cores
        self.sh2 = n2 // n_cores
        self.t0 = -(-self.sh1 // 128)
        self.t1 = -(-self.sh2 // 128)
        assert n_src0 % n_cores == 0
        self.bsh0 = n_src0 // n_cores      # S0 rows built per core
        self.bt0 = -(-self.bsh0 // 128)
        self.bsh1 = n1 // n_cores
        self.bt1 = -(-self.bsh1 // 128)
        self.rk = -(-self.nr2 // 8)        # rel K-chunk (125 for 1000)
        self.fsplit = min(128, hidden)


CFG = Cfg()


def prep_core_inputs(cfg, entity, relation, fc_w, fc_b, ids0, edge_src0,
                     etype0, ids1, edge_src1, etype1, core):
    """Host-side input prep for one core: sharding + index relayout +
    per-dst relation-count histogram (index statistics only)."""
    c = core
    d = cfg.d

    def shard_edges(arr, n_dst, sh, ntile):
        a = np.asarray(arr, np.int32).reshape(n_dst, d)[c * sh:(c + 1) * sh]
        pad = ntile * 128 - sh
        if pad:
            a = np.concatenate([a, np.zeros((pad, d), a.dtype)], 0)
        return a.reshape(ntile, 128, d)

    e0 = shard_edges(edge_src0, cfg.n1, cfg.sh1, cfg.t0)
    t0 = shard_edges(etype0, cfg.n1, cfg.sh1, cfg.t0)
    # remap block1 src ids to SA's half-block row ordering (core-major
    # within each half) so the AllGathers can run in halves
    es1 = np.asarray(edge_src1, np.int64)
    cg = es1 // cfg.sh1
    rg = es1 % cfg.sh1
    hsh = cfg.sh1 // 2
    es1_new = np.where(rg < hsh, cg * hsh + rg,
                       cfg.n1 // 2 + cg * hsh + (rg - hsh)).astype(np.int32)
    e1 = shard_edges(es1_new, cfg.n2, cfg.sh2, cfg.t1)
    t1 = shard_edges(etype1, cfg.n2, cfg.sh2, cfg.t1)

    # per-dst signed-relation count matrix for block0, transposed to
    # [T0, nr2, 128] (K-major for the PE), bf16 (counts <= d, exact)
    counts = np.zeros((cfg.t0, 128, cfg.nr2), np.float32)
    tiles_idx = np.arange(cfg.t0)[:, None, None]
    rows_idx = np.arange(128)[None, :, None]
    np.add.at(counts, (tiles_idx, rows_idx, t0), 1.0)
    c0rT = np.ascontiguousarray(counts.transpose(0, 2, 1), np.float32)

    def build_ids(idvec, nrows, ntiles):
        padded = np.zeros(ntiles * 128, np.int32)
        padded[:nrows] = np.asarray(idvec, np.int32)
        return padded.reshape(ntiles, 128, 1)

    ids0 = np.asarray(ids0)
    ids1 = np.asarray(ids1)
    ids0_sh = ids0[c * cfg.bsh0:(c + 1) * cfg.bsh0]
    ids1_sh = ids1[c * cfg.bsh1:(c + 1) * cfg.bsh1]

    return dict(
        entity=np.ascontiguousarray(entity, np.float32),
        relation=np.ascontiguousarray(relation, np.float32),
        fc_w=np.ascontiguousarray(fc_w, np.float32),
        fc_b=np.ascontiguousarray(fc_b, np.float32).reshape(cfg.nt, 1),
        i_e0=e0, i_t0=t0, i_e1=e1, i_t1=t1,
        c0rT=c0rT,
        b_ids0=build_ids(ids0_sh, cfg.bsh0, cfg.bt0),
        b_ids1=build_ids(ids1_sh, cfg.bsh1, cfg.bt1),
    )


def _bc(ap_obj, dims):
    """Manual AP with given free dims (for step-0 broadcasts)."""
    return bass.AP(ap_obj.tensor, ap_obj.offset, [ap_obj.ap[0]] + dims)


def build_program(cfg):
    nc = bacc.Bacc("TRN2", target_bir_lowering=False, debug=False,
                   num_devices=cfg.n_cores)
    d, h, nt, L = cfg.d, cfg.h, cfg.nt, cfg.l
    H5, DH = cfg.heads, cfg.dh
    fA = cfg.fsplit
    fB = h - fA
    groups = [list(range(g, min(g + 4, L))) for g in range(0, L, 4)]
    ncols = L * 128
    nch = [(j, min(512, ncols - j)) for j in range(0, ncols, 512)]
    pos = [(i, min(125, cfg.n_rels - i)) for i in range(0, cfg.n_rels, 125)]
    rchunks = pos + [(cfg.n_rels + i, r) for i, r in pos]

    ein = lambda name, shape, dt: nc.dram_tensor(name, shape, dt,
                                                 kind="ExternalInput").ap()
    entity = ein("entity", [cfg.n_ent, h], F32)
    relation = ein("relation", [cfg.n_rels, h], F32)
    fc_w = ein("fc_w", [nt, h], F32)
    fc_b = ein("fc_b", [nt, 1], F32)
    i_e0 = ein("i_e0", [cfg.t0, 128, d], I32)
    i_t0 = ein("i_t0", [cfg.t0, 128, d], I32)
    i_e1 = ein("i_e1", [cfg.t1, 128, d], I32)
    i_t1 = ein("i_t1", [cfg.t1, 128, d], I32)
    c0rT = ein("c0rT", [cfg.t0, cfg.nr2, 128], F32)
    b_ids0 = ein("b_ids0", [cfg.bt0, 128, 1], I32)
    b_ids1 = ein("b_ids1", [cfg.bt1, 128, 1], I32)

    out = nc.dram_tensor("out", [cfg.sh2, nt], F32, kind="ExternalOutput").ap()

    with tile.TileContext(nc) as tc:
        with tc.tile_pool(name="dram", bufs=1, space="DRAM") as dram, \
             tc.tile_pool(name="const", bufs=1) as cst, \
             tc.tile_pool(name="mail", bufs=2) as mailp, \
             tc.tile_pool(name="xt", bufs=2) as xtp, \
             tc.tile_pool(name="small", bufs=2) as smp, \
             tc.tile_pool(name="psT", bufs=2, space="PSUM") as psTp, \
             tc.tile_pool(name="psF", bufs=2, space="PSUM") as psFp, \
             tc.tile_pool(name="psP", bufs=2, space="PSUM") as psPp:

            # ---------------- DRAM internals ----------------
            S0 = dram.tile([cfg.n_src0, h], BF16)         # entity[ids0]
            SA = dram.tile([cfg.n1, 2 * h], BF16)         # [S1 | agg]
            rel2 = dram.tile([cfg.nr2, h], BF16)          # [relation; -relation]
            aggp = dram.tile([cfg.n2, h], BF16)
            shS0 = dram.tile([cfg.bsh0, h], BF16)
            shS1 = dram.tile([cfg.bsh1, h], BF16)
            shAgg = dram.tile([cfg.sh1, h], BF16)
            S1g = dram.tile([cfg.n1, h], BF16)

            # ---------------- constants ----------------
            id_f32 = cst.tile([128, 128], F32)
            make_identity(nc, id_f32[:])
            id_bf = cst.tile([128, 128], BF16)
            nc.vector.tensor_copy(id_bf[:], id_f32[:])

            fcw_sb = cst.tile([nt, h], F32)
            nc.sync.dma_start(fcw_sb[:], fc_w[:, :])
            fcb_sb = cst.tile([nt, 1], F32)
            nc.sync.dma_start(fcb_sb[:], fc_b[:, :])
            fcw_a = cst.tile([fA, nt], BF16)
            fcw_b = cst.tile([fB, nt], BF16)
            pw = psFp.tile([128, 512], F32, tag="psF")
            nc.tensor.transpose(pw[:fA, :nt], fcw_sb[:, 0:fA], id_f32[:nt, :nt])
            nc.vector.tensor_copy(fcw_a[:], pw[:fA, :nt])
            pw2 = psFp.tile([128, 512], F32, tag="psF")
            nc.tensor.transpose(pw2[:fB, :nt], fcw_sb[:, fA:h], id_f32[:nt, :nt])
            nc.vector.tensor_copy(fcw_b[:], pw2[:fB, :nt])

            # rel2 build: [relation; -relation], plus SBUF chunks (K-major,
            # 125 rows each, positives then negatives) for the block0 matmul
            pos_chunks = []
            neg_chunks = []
            for i in range(0, cfg.n_rels, 125):
                rows = min(125, cfg.n_rels - i)
                rl = cst.tile([128, h], F32, name=f"relsb{i}")
                nc.sync.dma_start(rl[:rows, :], relation[i:i + rows, :])
                rlb = smp.tile([128, h], BF16, tag="relbf")
                nc.vector.tensor_copy(rlb[:rows, :], rl[:rows, :])
                nc.sync.dma_start(rel2[i:i + rows, :], rlb[:rows, :])
                rln = cst.tile([128, h], F32, name=f"relsbn{i}")
                nc.vector.tensor_scalar_mul(rln[:rows, :], rl[:rows, :], -1.0)
                rlnb = smp.tile([128, h], BF16, tag="relbf")
                nc.vector.tensor_copy(rlnb[:rows, :], rln[:rows, :])
                nc.sync.dma_start(rel2[cfg.n_rels + i:cfg.n_rels + i + rows, :],
                                  rlnb[:rows, :])
                pos_chunks.append((rl, rows))
                neg_chunks.append((rln, rows))
            rel_chunks = pos_chunks + neg_chunks

            # ---------------- src pool builds (sharded) ----------------
            def build_pool(ids_arr, ntiles, nrows, shard):
                for t in range(ntiles):
                    rows = min(128, nrows - t * 128)
                    it = smp.tile([128, 1], I32, tag="bidx")
                    nc.sync.dma_start(it[:], ids_arr[t, :, :])
                    g = smp.tile([128, h], F32, tag="brow")
                    nc.gpsimd.indirect_dma_start(
                        out=g[:], out_offset=None, in_=entity[:, :],
                        in_offset=IndirectOffsetOnAxis(ap=it[:], axis=0))
                    gb = smp.tile([128, h], BF16, tag="browb")
                    nc.vector.tensor_copy(gb[:], g[:])
                    nc.sync.dma_start(shard[t * 128:t * 128 + rows, :],
                                      gb[:rows, :])

            build_pool(b_ids0, cfg.bt0, cfg.bsh0, shS0)
            build_pool(b_ids1, cfg.bt1, cfg.bsh1, shS1)

            grp = [list(range(cfg.n_cores))]
            nc.gpsimd.collective_compute("AllGather", mybir.AluOpType.bypass,
                                         replica_groups=grp, ins=[shS0.opt()],
                                         outs=[S0.opt()])
            # S1 AllGather in halves; SA rows use "half-block" ordering
            # (core-major within each half) — edge ids are host-remapped to
            # match. Copies into SA's strided layout are <16k descs each.
            hw_ = cfg.n1 // 2
            hsh1 = cfg.bsh1 // 2
            nc.gpsimd.collective_compute("AllGather", mybir.AluOpType.bypass,
                                         replica_groups=grp,
                                         ins=[shS1[0:hsh1, :].opt()],
                                         outs=[S1g[0:hw_, :].opt()])
            nc.sync.dma_start(SA[0:hw_, 0:h], S1g[0:hw_, :])
            nc.gpsimd.collective_compute("AllGather", mybir.AluOpType.bypass,
                                         replica_groups=grp,
                                         ins=[shS1[hsh1:cfg.bsh1, :].opt()],
                                         outs=[S1g[hw_:cfg.n1, :].opt()])
            nc.sync.dma_start(SA[hw_:cfg.n1, 0:h], S1g[hw_:cfg.n1, :])

            # ---------------- block 1 rel prefetch ----------------
            # rel2 is ready before the AllGathers; emitting the first few
            # block-1 rel mailbox gathers here lets the Pool engine fill the
            # S0-AllGather and agg-AllGather wait windows instead of idling.
            PREF = 5
            RELBUFS = 7

            def emit_rel(t):
                it1 = smp.tile([128, d], I32, tag="it1")
                nc.sync.dma_start(it1[:], i_t1[t, :, :])
                rl = mailp.tile([128, d * h], BF16, tag="mailR", bufs=RELBUFS)
                for k in range(d):
                    nc.gpsimd.indirect_dma_start(
                        out=rl[:, k * h:(k + 1) * h], out_offset=None,
                        in_=rel2[:, :],
                        in_offset=IndirectOffsetOnAxis(ap=it1[:, k:k + 1], axis=0))
                return rl

            rel_q = [emit_rel(t) for t in range(min(PREF, cfg.t1))]

            # ---------------- block 0 ----------------
            for t in range(cfg.t0):
                rows = min(128, cfg.sh1 - t * 128)
                ie = smp.tile([128, d], I32, tag="ie0")
                nc.sync.dma_start(ie[:], i_e0[t, :, :])
                mb = mailp.tile([128, d * h], BF16, tag="mailSA", bufs=3)
                for k in range(d):
                    nc.gpsimd.indirect_dma_start(
                        out=mb[:, k * h:(k + 1) * h], out_offset=None,
                        in_=S0[:, :],
                        in_offset=IndirectOffsetOnAxis(ap=ie[:, k:k + 1], axis=0))
                # relation sum via count matmul: psr = C^T.T @ rel2_sb
                psr = psPp.tile([128, 200], F32, tag="psP")
                for ci, (r0, rn_) in enumerate(rchunks):
                    ct = smp.tile([128, 128], F32, tag="c0chunk")
                    nc.sync.dma_start(ct[:rn_, :], c0rT[t, r0:r0 + rn_, :])
                    rtile, rrows = rel_chunks[ci]
                    assert rrows == rn_
                    nc.tensor.matmul(psr[:, :h], lhsT=ct[:rn_, :],
                                     rhs=rtile[:rn_, :],
                                     start=(ci == 0), stop=(ci == len(rchunks) - 1))
                su = smp.tile([128, h], F32, tag="msum")
                mrv = mb[:].rearrange("p (k e) -> p e k", k=d, e=h)
                nc.vector.tensor_reduce(su[:], mrv, axis=mybir.AxisListType.X,
                                        op=mybir.AluOpType.add)
                abf = smp.tile([128, h], F32, tag="aggf")
                nc.vector.tensor_add(abf[:], su[:], psr[:, :h])
                ab = smp.tile([128, h], BF16, tag="aggbf")
                nc.vector.tensor_scalar_mul(ab[:], abf[:], 1.0 / d)
                nc.sync.dma_start(shAgg[t * 128:t * 128 + rows, :], ab[:rows, :])

            # split agg AllGather + SA copy into halves so the half-1 copy
            # overlaps the half-2 AllGather (shorter time-to-SA-ready)
            hsh = cfg.sh1 // 2
            nc.gpsimd.collective_compute("AllGather", mybir.AluOpType.bypass,
                                         replica_groups=grp,
                                         ins=[shAgg[0:hsh, :].opt()],
                                         outs=[aggp[0:hw_, :].opt()])
            nc.sync.dma_start(SA[0:hw_, h:2 * h], aggp[0:hw_, :])
            nc.gpsimd.collective_compute("AllGather", mybir.AluOpType.bypass,
                                         replica_groups=grp,
                                         ins=[shAgg[hsh:cfg.sh1, :].opt()],
                                         outs=[aggp[hw_:cfg.n1, :].opt()])
            nc.sync.dma_start(SA[hw_:cfg.n1, h:2 * h], aggp[hw_:cfg.n1, :])

            # ---------------- block 1 ----------------
            scale_q = (1.0 / L) * (DH ** -0.5)
            for t in range(cfg.t1):
                rows = min(128, cfg.sh2 - t * 128)
                ie = smp.tile([128, d], I32, tag="ie1")
                nc.sync.dma_start(ie[:], i_e1[t, :, :])

                sa = mailp.tile([128, d * 2 * h], BF16, tag="mailSA", bufs=3)
                for k in range(d):
                    nc.gpsimd.indirect_dma_start(
                        out=sa[:, k * 2 * h:(k + 1) * 2 * h], out_offset=None,
                        in_=SA[:, :],
                        in_offset=IndirectOffsetOnAxis(ap=ie[:, k:k + 1], axis=0))
                if t + PREF < cfg.t1:
                    rel_q.append(emit_rel(t + PREF))
                rl = rel_q[t]

                # msg/aggm halves += rel
                rv = rl[:].rearrange("p (k e) -> p k e", k=d, e=h)
                sav = sa[:].rearrange("p (k s e) -> p k s e", k=d, s=2, e=h)
                nc.vector.tensor_add(sav[:, :, 0, :], sav[:, :, 0, :], rv)
                nc.vector.tensor_add(sav[:, :, 1, :], sav[:, :, 1, :], rv)

                p_tiles = []
                for s in range(2):          # 0 = msg, 1 = aggm
                    # mean slot
                    su32 = smp.tile([128, h], F32, tag="msum")
                    mrv = sa[:].rearrange("p (k s e) -> p s e k", k=d, s=2, e=h)[:, s, :, :]
                    nc.vector.tensor_reduce(su32[:], mrv, axis=mybir.AxisListType.X,
                                            op=mybir.AluOpType.add)
                    su = smp.tile([128, h], BF16, tag="msumb")
                    nc.vector.tensor_copy(su[:], su32[:])

                    xA = xtp.tile([fA, ncols], BF16, tag="xA")
                    xB = xtp.tile([fB, ncols], BF16, tag="xB")
                    for g in groups:
                        pst = psTp.tile([128, 1024], BF16, tag="psT")
                        for j, l in enumerate(g):
                            if l < d:
                                c0 = l * 2 * h + s * h
                                srcA = sa[:, c0:c0 + fA]
                                srcB = sa[:, c0 + fA:c0 + h]
                            else:
                                srcA = su[:, 0:fA]
                                srcB = su[:, fA:h]
                            nc.tensor.transpose(pst[:, 256 * j:256 * j + 128],
                                                srcA, id_bf[:128, :128])
                            nc.tensor.transpose(
                                pst[:fB, 256 * j + 128:256 * j + 256],
                                srcB, id_bf[:128, :128])
                        n_g = len(g)
                        l0 = g[0]
                        sc = (1.0 / d) if l0 == d else 1.0
                        inA = pst[:].rearrange("p (b c) -> p b c", b=4, c=256)[:, 0:n_g, 0:128]
                        outA = xA[:].rearrange("f (l n) -> f l n", l=L, n=128)[:, l0:l0 + n_g, :]
                        nc.scalar.activation(outA, inA,
                                             mybir.ActivationFunctionType.Relu,
                                             scale=sc)
                        inB = pst[:fB].rearrange("p (b c) -> p b c", b=4, c=256)[:, 0:n_g, 128:256]
                        outB = xB[:].rearrange("f (l n) -> f l n", l=L, n=128)[:, l0:l0 + n_g, :]
                        nc.scalar.activation(outB, inB,
                                             mybir.ActivationFunctionType.Relu,
                                             scale=sc)

                    pf = smp.tile([nt, ncols], BF16, tag="pf")
                    for j0, w in nch:
                        psf = psFp.tile([128, 512], F32, tag="psF")
                        nc.tensor.matmul(psf[:nt, :w], lhsT=fcw_a[:],
                                         rhs=xA[:, j0:j0 + w], start=True,
                                         stop=False)
                        nc.tensor.matmul(psf[:nt, :w], lhsT=fcw_b[:],
                                         rhs=xB[:, j0:j0 + w], start=False,
                                         stop=True)
                        nc.scalar.activation(pf[:, j0:j0 + w], psf[:nt, :w],
                                             mybir.ActivationFunctionType.Identity,
                                             bias=fcb_sb[:])

                    p_s = smp.tile([128, L * nt], BF16, tag=f"p{s}")
                    for g in groups:
                        psp = psPp.tile([128, 200], BF16, tag="psP")
                        for j, l in enumerate(g):
                            nc.tensor.transpose(psp[:, nt * j:nt * j + nt],
                                                pf[:, 128 * l:128 * l + 128],
                                                id_bf[:nt, :nt])
                        n_g = len(g)
                        l0 = g[0]
                        inP = psp[:].rearrange("p (b c) -> p b c", b=4, c=nt)[:, 0:n_g, :]
                        outP = p_s[:].rearrange("p (c l) -> p l c", c=nt, l=L)[:, l0:l0 + n_g, :]
                        nc.vector.tensor_copy(outP, inP)
                    p_tiles.append(p_s)

                # ---- MHA pooling + combine ----
                outs_f = []
                for p_s in p_tiles:
                    pap = p_s[:]
                    pv = pap.rearrange("p (h dd l) -> p h dd l", h=H5, dd=DH, l=L)
                    q32 = smp.tile([128, nt], F32, tag="q32")
                    qv = pap.rearrange("p (c l) -> p c l", c=nt, l=L)
                    nc.vector.tensor_reduce(q32[:], qv, axis=mybir.AxisListType.X,
                                            op=mybir.AluOpType.add)
                    qs = smp.tile([128, nt], BF16, tag="qs")
                    nc.vector.tensor_scalar_mul(qs[:], q32[:], scale_q)
                    sc1 = smp.tile([128, L * nt], BF16, tag="sc1")
                    sc1v = sc1[:].rearrange("p (h dd l) -> p h dd l", h=H5, dd=DH, l=L)
                    qsv = _bc(qs[:], [[DH, H5], [1, DH], [0, L]])
                    nc.vector.tensor_mul(sc1v, pv, qsv)
                    scor = smp.tile([128, H5 * L], F32, tag="scor")
                    sc1r = sc1[:].rearrange("p (h dd l) -> p h l dd", h=H5, dd=DH, l=L)
                    scv = scor[:].rearrange("p (h l) -> p h l", h=H5, l=L)
                    nc.vector.tensor_reduce(scv, sc1r, axis=mybir.AxisListType.X,
                                            op=mybir.AluOpType.add)
                    smax = smp.tile([128, H5], F32, tag="smax")
                    nc.vector.tensor_reduce(smax[:], scv, axis=mybir.AxisListType.X,
                                            op=mybir.AluOpType.max)
                    smaxb = _bc(smax[:], [[1, H5], [0, L]])
                    nc.vector.tensor_sub(scv, scv, smaxb)
                    nc.scalar.activation(scor[:], scor[:],
                                         mybir.ActivationFunctionType.Exp)
                    ssum = smp.tile([128, H5], F32, tag="ssum")
                    nc.vector.tensor_reduce(ssum[:], scv, axis=mybir.AxisListType.X,
                                            op=mybir.AluOpType.add)
                    rinv = smp.tile([128, H5], F32, tag="rinv")
                    nc.vector.reciprocal(rinv[:], ssum[:])
                    sc2 = smp.tile([128, L * nt], BF16, tag="sc2")
                    sc2v = sc2[:].rearrange("p (h dd l) -> p h dd l", h=H5, dd=DH, l=L)
                    scb = _bc(scor[:], [[L, H5], [0, DH], [1, L]])
                    nc.vector.tensor_mul(sc2v, pv, scb)
                    asum = smp.tile([128, nt], F32, tag="asum")
                    sc2r = sc2[:].rearrange("p (c l) -> p c l", c=nt, l=L)
                    nc.vector.tensor_reduce(asum[:], sc2r, axis=mybir.AxisListType.X,
                                            op=mybir.AluOpType.add)
                    of = smp.tile([128, nt], F32, tag="of")
                    rinvb = _bc(rinv[:], [[1, H5], [0, DH]])
                    nc.vector.tensor_mul(of[:], asum[:], rinvb)
                    outs_f.append(of)

                pred = smp.tile([128, nt], F32, tag="pred")
                nc.vector.tensor_scalar_mul(pred[:], outs_f[1][:], 1.0 - cfg.beta)
                nc.vector.scalar_tensor_tensor(
                    pred[:], outs_f[0][:], cfg.beta, pred[:],
                    op0=mybir.AluOpType.mult, op1=mybir.AluOpType.add)
                osb = smp.tile([128, nt], F32, tag="osb")
                nc.scalar.activation(osb[:], pred[:],
                                     mybir.ActivationFunctionType.Sigmoid)
                nc.sync.dma_start(out[t * 128:t * 128 + rows, :], osb[:rows, :])

    nc.compile()
    return nc


_CACHE = {}


def _get_program():
    if "nc" not in _CACHE:
        _CACHE["nc"] = build_program(CFG)
    return _CACHE["nc"]


def kernel(entity, relation, fc_w, fc_b, ids0, edge_src0, etype0, ids1,
           edge_src1, etype1):
    cfg = CFG
    nc = _get_program()
    in_maps = [
        prep_core_inputs(cfg, entity, relation, fc_w, fc_b, ids0, edge_src0,
                         etype0, ids1, edge_src1, etype1, c)
        for c in range(cfg.n_cores)
    ]
    res = run_bass_kernel_spmd(nc, in_maps, core_ids=list(range(cfg.n_cores)),
                               trace=False)
    out = np.concatenate([res.results[c]["out"] for c in range(cfg.n_cores)],
                         axis=0)
    return out.astype(np.float32)

